# revision 35
# baseline (speedup 1.0000x reference)
"""Trainium2 Bass kernel for nn_GatedLinear (gated LoRA-MoE linear layer).

Math (see reference):
  base_out = x @ base_w.T + base_b
  logits   = x @ router_w.T ; top-2 softmax -> dense per-expert gate
  h        = x @ lora_A.T   ; rank_w = repeat(gate*scalings, 16)
  out      = base_out + (h * rank_w) @ lora_B.T

Sharding: pure data-parallel over batch*seq across 8 cores (1024 tokens
per core); all weights replicated. No collectives.

Implementation notes:
- The dominant base matmul runs in fp8 e4m3 with DoubleRow perf mode
  (2 k-subtiles contracted per instruction at 0.5 cycles/row). base_w is
  prescaled by 64 on the host so its values sit in e4m3's normal range;
  the 1/64 is folded into the bias epilogue. lora_B is prescaled by 64
  too so its f32r accumulation step shares the same PSUM scale.
  Measured end-to-end rel err of this scheme on the reference seed is
  ~1.1e-2 (tolerance 2e-2).
- The router must match fp32 top-2 selection exactly, so logits use
  true-fp32 matmuls on a separate fp32 copy of x, streamed per 256-token
  gating tile. The lora path (lora_A/lora_B matmuls, gated activations)
  runs in bf16 (same PE rate as f32r, half the SBUF/DMA, ~2e-3 error),
  fed by a second small bf16 x stream.
- All DRAM layouts are host-swizzled so every DMA is large-contiguous
  per partition (the previous version streamed base_w as 512-byte
  descriptors, starving the PE).
- Router/gating work for gating tile g+1 is interleaved with the gate
  expansion of tile g so the tensor engine never waits on the vector
  engine's top-k chain; the fused per-output-tile loop follows.
- Output is written bf16 in a DMA-friendly layout and de-swizzled/upcast
  on the host.
"""

import numpy as np
import ml_dtypes


def _ensure_path():
    try:
        import concourse.bass  # noqa: F401
    except ImportError:
        import sys

        for p in ("/opt/trn_rl_repo", "/root/.axon_site/_ro/trn_rl_repo"):
            if p not in sys.path:
                sys.path.insert(0, p)


N_CORES = 8
B, S, D, O = 4, 2048, 4096, 4096
T = B * S              # 8192 tokens total
T_PC = T // N_CORES    # 1024 tokens per core
E = 8                  # experts
RANK = 16
R = E * RANK           # 128 fused rank dim
P = 128
KO = D // P            # 32 k-subtiles of the contraction dim
KO2 = KO // 2          # 16 DoubleRow k-pairs
OT = O // P            # 32 output-feature tiles
TTILE = 512            # tokens per base-matmul moving operand
NT = T_PC // TTILE     # 2 token tiles per core
GT = 256               # gating token-tile size
NGT = T_PC // GT       # 4 gating tiles per core
NGC = GT // P          # 128-token chunks per gating tile
WS = 64.0              # fp8 weight prescale

_prog_cache = {}


def _build_program():
    """Build the single-core SPMD Bass program (same on all 8 cores)."""
    _ensure_path()
    import concourse.mybir as mybir
    import concourse.tile as tile
    from concourse import bacc

    f32 = mybir.dt.float32
    f32r = mybir.dt.float32r
    f8 = mybir.dt.float8e4
    bf16 = mybir.dt.bfloat16
    Alu = mybir.AluOpType
    Act = mybir.ActivationFunctionType
    DRow = mybir.MatmulPerfMode.DoubleRow

    nc = bacc.Bacc(
        "TRN2",
        target_bir_lowering=False,
        debug=False,
        num_devices=N_CORES,
    )

    x8d = nc.dram_tensor("x8", [P, KO, T_PC], f8, kind="ExternalInput").ap()
    xfd = nc.dram_tensor("xf", [NGT, P, KO, GT], f32, kind="ExternalInput").ap()
    xbd = nc.dram_tensor("xb", [NGT, P, KO, GT], bf16, kind="ExternalInput").ap()
    w8d = nc.dram_tensor("w8", [OT, P, KO, P], f8, kind="ExternalInput").ap()
    aAd = nc.dram_tensor("aa", [P, KO, R], bf16, kind="ExternalInput").ap()
    bBd = nc.dram_tensor("blo", [P, OT, P], bf16, kind="ExternalInput").ap()
    rtd = nc.dram_tensor("rt", [P, KO, E], f32, kind="ExternalInput").ap()
    bbd = nc.dram_tensor("bias", [P, OT], f32, kind="ExternalInput").ap()
    e8d = nc.dram_tensor("e8", [E, P], f32, kind="ExternalInput").ap()
    idd = nc.dram_tensor("idm", [P, P], f32, kind="ExternalInput").ap()
    yod = nc.dram_tensor("yo", [OT, NT, P, TTILE], bf16, kind="ExternalOutput").ap()

    from contextlib import ExitStack

    with tile.TileContext(nc) as tc:
        with (
            tc.tile_pool(name="pp", bufs=1) as pp,
            tc.tile_pool(name="xfp", bufs=2) as xfp,
            tc.tile_pool(name="wp", bufs=3) as wp,
            tc.tile_pool(name="gp", bufs=2) as gp,
            tc.tile_pool(name="ob", bufs=3) as ob,
        ):
            # PSUM pools: lg 1 + h 2 + sm 3 (tp/tp2/rg) + acc 2 = exactly
            # the 8 banks. Keeping them all open lets the scheduler overlap
            # fused-loop matmuls into late-gating PE gaps.
            phase1 = ExitStack()
            ps_lg = phase1.enter_context(
                tc.tile_pool(name="ps_lg", bufs=1, space="PSUM")
            )
            ps_h = phase1.enter_context(
                tc.tile_pool(name="ps_h", bufs=2, space="PSUM")
            )
            ps_sm = phase1.enter_context(
                tc.tile_pool(name="ps_sm", bufs=1, space="PSUM")
            )

            # ---- resident constants; small ones on the scalar queue ----
            rtsb = pp.tile([P, KO, E], f32)
            nc.scalar.dma_start(rtsb[:], rtd[:])
            aAsb = pp.tile([P, KO, R], bf16)
            nc.scalar.dma_start(aAsb[:], aAd[:])
            bbsb = pp.tile([P, OT], f32)
            nc.scalar.dma_start(bbsb[:], bbd[:])
            e8sb = pp.tile([E, P], f32)
            nc.scalar.dma_start(e8sb[:], e8d[:])
            idsb = pp.tile([P, P], f32)
            nc.scalar.dma_start(idsb[:], idd[:])
            bBsb = pp.tile([P, OT, P], bf16)  # DMA'd mid-gating (decongest)

            hwsb = pp.tile([P, T_PC], bf16)  # gated rank activations [r, t]
            x8sb = pp.tile([P, KO, T_PC], f8)

            # gpsimd queue: first two gating x-slices (fp32 + bf16), then
            # the fp8 x, then the rest (pool pacing keeps the queue from
            # running far ahead; x8 is ordered before xf[2] so it is not
            # blocked by the xf pool semaphore).
            xft = [None] * NGT
            xbt = [None] * NGT

            def issue_xf(g, chunks=1, bufs=3):
                xft[g] = xfp.tile(
                    [P, KO, GT], f32, tag="xf", name=f"xf{g}", bufs=bufs
                )
                # finer chunks give finer dep granularity: the first logits
                # matmuls can start as soon as their ko-slices arrive
                ck = KO // chunks
                for c in range(chunks):
                    nc.gpsimd.dma_start(
                        xft[g][:, c * ck : (c + 1) * ck, :],
                        xfd[g, :, c * ck : (c + 1) * ck, :],
                    )

            def issue_xb(g):
                xbt[g] = xfp.tile(
                    [P, KO, GT], bf16, tag="xb", name=f"xb{g}", bufs=2
                )
                nc.gpsimd.dma_start(xbt[g][:], xbd[g])

            issue_xf(0, chunks=8)
            issue_xb(0)
            issue_xf(1)
            issue_xb(1)
            issue_xf(2)
            issue_xb(2)
            issue_xf(3)
            issue_xb(3)
            # x8 is only needed by the fused loop (~120us in); keep it
            # behind the gating-phase x streams
            nc.gpsimd.dma_start(x8sb[:], x8d[:])

            # ---- gating round helpers (issue-order is the schedule) ----
            def issue_lg(g):
                """Router logits for gating tile g: exact fp32, expert-major.

                router_w is the stationary operand (only 8 columns, so
                LDWEIGHTS is ~7ns and hides); a stationary-x variant was
                measured slower (each 128-col fp32 weight load cannot hide
                behind an 8-row moving pass).
                """
                lg = ps_lg.tile([E, GT], f32, tag="lg")
                for ko in range(KO):
                    nc.tensor.matmul(
                        lg[:],
                        lhsT=rtsb[:, ko, :],
                        rhs=xft[g][:, ko, :],
                        start=(ko == 0),
                        stop=(ko == KO - 1),
                    )
                lgs = gp.tile([E, GT], f32, tag="lgs")
                nc.vector.tensor_copy(lgs[:], lg[:])
                return lgs

            def issue_tpfw(g, lgs):
                """Transpose logits to token-major [tok, chunk, expert]."""
                ltk = gp.tile([P, NGC, E], f32, tag="ltk")
                tp = ps_sm.tile([P, NGC, E], f32, tag="tp")
                for c in range(NGC):
                    nc.tensor.transpose(
                        tp[:, c, :], lgs[:, c * P : (c + 1) * P], idsb[:E, :E]
                    )
                nc.vector.tensor_copy(ltk[:], tp[:])
                return ltk

            def issue_h(g):
                """lora_A rank activations for tile g (bf16)."""
                hps = ps_h.tile([P, GT], f32, tag="h")
                for ko in range(KO):
                    nc.tensor.matmul(
                        hps[:],
                        lhsT=aAsb[:, ko, :],
                        rhs=xbt[g][:, ko, :],
                        start=(ko == 0),
                        stop=(ko == KO - 1),
                    )
                return hps

            def issue_topk(g, ltk):
                """Top-2 + softmax along the expert axis (DVE only)."""
                m1 = gp.tile([P, NGC, 1], f32, tag="m1")
                nc.vector.tensor_reduce(m1[:], ltk[:], mybir.AxisListType.X, Alu.max)
                mask1 = gp.tile([P, NGC, E], f32, tag="mask1")
                nc.vector.tensor_tensor(
                    mask1[:], ltk[:], m1.to_broadcast((P, NGC, E)), Alu.is_equal
                )
                l2 = gp.tile([P, NGC, E], f32, tag="l2")
                nc.vector.scalar_tensor_tensor(
                    l2[:], mask1[:], -1e30, ltk[:], Alu.mult, Alu.add
                )
                m2 = gp.tile([P, NGC, 1], f32, tag="m2")
                nc.vector.tensor_reduce(m2[:], l2[:], mybir.AxisListType.X, Alu.max)
                mask2 = gp.tile([P, NGC, E], f32, tag="mask2")
                nc.vector.tensor_tensor(
                    mask2[:], l2[:], m2.to_broadcast((P, NGC, E)), Alu.is_equal
                )
                dlt = gp.tile([P, NGC, 1], f32, tag="dlt")
                nc.vector.tensor_tensor(dlt[:], m2[:], m1[:], Alu.subtract)
                g2 = gp.tile([P, NGC, 1], f32, tag="g2")
                nc.scalar.activation(g2[:], dlt[:], Act.Sigmoid)
                g1 = gp.tile([P, NGC, 1], f32, tag="g1")
                nc.vector.tensor_scalar(g1[:], g2[:], -1.0, 1.0, Alu.mult, Alu.add)
                gate = gp.tile([P, NGC, E], f32, tag="gate", bufs=3)
                nc.vector.tensor_tensor(
                    gate[:], mask1[:], g1.to_broadcast((P, NGC, E)), Alu.mult
                )
                gm2 = gp.tile([P, NGC, E], f32, tag="gm2")
                nc.vector.tensor_tensor(
                    gm2[:], mask2[:], g2.to_broadcast((P, NGC, E)), Alu.mult
                )
                nc.vector.tensor_tensor(gate[:], gate[:], gm2[:], Alu.add)
                return gate

            def issue_expand(g, gate, hps):
                """Gates back to expert-major, expand to rank slots, gate h."""
                ts = slice(g * GT, (g + 1) * GT)
                gts = gp.tile([E, NGC, P], f32, tag="gts")
                tp2 = ps_sm.tile([E, NGC, P], f32, tag="tp2")
                for c in range(NGC):
                    nc.tensor.transpose(tp2[:, c, :], gate[:, c, :], idsb[:])
                nc.vector.tensor_copy(gts[:], tp2[:])
                RG = ps_sm.tile([P, GT], f32, tag="rg")
                nc.tensor.matmul(
                    RG[:], lhsT=e8sb[:], rhs=gts[:], start=True, stop=True
                )
                rgs = gp.tile([P, GT], f32, tag="rgs")
                nc.vector.tensor_copy(rgs[:], RG[:])
                nc.vector.tensor_tensor(hwsb[:, ts], hps[:], rgs[:], Alu.mult)

            # ---- gating rounds, software-pipelined ----
            # Per round: PE does lg(g), h(g), tpfw(g); the expand (tpbk,
            # e8, hw-gating) of round g-2 follows, two rounds behind, so
            # no PE op ever waits on the DVE top-k chain (measured
            # ~2.4us/round stall with a one-round lag). Mid-round we also
            # kick DMAs that are only needed later (lora_B, first base_w
            # tiles) to keep the startup window clear for the x streams.
            hps = [None] * NGT
            gates = [None] * NGT
            wtiles = []
            for g in range(NGT):
                lgsg = issue_lg(g)
                # expand(g-2) is issued BEFORE h(g) so the ps_h buffer
                # rotation (bufs=2) sees the g-2 consumer at alloc time
                if g >= 2:
                    issue_expand(g - 2, gates[g - 2], hps[g - 2])
                hps[g] = issue_h(g)
                ltkg = issue_tpfw(g, lgsg)
                gates[g] = issue_topk(g, ltkg)
                if g == 1:
                    nc.scalar.dma_start(bBsb[:], bBd[:])
                if g == 2:
                    for k in range(3):
                        wsb = wp.tile([P, KO, P], f8, tag="w", name=f"wpre{k}")
                        nc.scalar.dma_start(wsb[:], w8d[k])
                        wtiles.append(wsb)
            issue_expand(NGT - 2, gates[NGT - 2], hps[NGT - 2])
            issue_expand(NGT - 1, gates[NGT - 1], hps[NGT - 1])

            # ---- fused base + lora output loop ----
            with tc.tile_pool(name="ps_acc", bufs=2, space="PSUM") as ps_acc:
                for ot in range(OT):
                    if ot < len(wtiles):
                        wsb = wtiles[ot]
                    else:
                        wsb = wp.tile([P, KO, P], f8, tag="w")
                        nc.sync.dma_start(wsb[:], w8d[ot])
                    for tt in range(NT):
                        ts = slice(tt * TTILE, (tt + 1) * TTILE)
                        acc = ps_acc.tile([P, TTILE], f32, tag="acc")
                        for k2 in range(KO2):
                            nc.tensor.matmul(
                                acc[:],
                                lhsT=wsb[:, 2 * k2 : 2 * k2 + 2, :],
                                rhs=x8sb[:, 2 * k2 : 2 * k2 + 2, ts],
                                start=(k2 == 0),
                                stop=False,
                                perf_mode=DRow,
                            )
                        nc.tensor.matmul(
                            acc[:],
                            lhsT=bBsb[:, ot, :],
                            rhs=hwsb[:, ts],
                            start=False,
                            stop=True,
                        )
                        osb = ob.tile([P, TTILE], bf16, tag="osb")
                        nc.vector.scalar_tensor_tensor(
                            osb[:],
                            acc[:],
                            1.0 / WS,
                            bbsb[:, ot, None].to_broadcast((P, TTILE)),
                            Alu.mult,
                            Alu.add,
                        )
                        nc.scalar.dma_start(yod[ot, tt], osb[:])
            phase1.close()

    nc.compile()
    return nc


def get_program():
    if "nc" not in _prog_cache:
        _prog_cache["nc"] = _build_program()
    return _prog_cache["nc"]


def make_in_maps(x, base_w, base_b, lora_A, lora_B, router_w, scalings):
    """Host-side sharding/layout prep -> per-core input dicts."""
    f8 = ml_dtypes.float8_e4m3
    x = np.ascontiguousarray(np.asarray(x, dtype=np.float32).reshape(T, D))
    base_w = np.asarray(base_w, dtype=np.float32)
    base_b = np.asarray(base_b, dtype=np.float32)
    lora_A = np.asarray(lora_A, dtype=np.float32)
    lora_B = np.asarray(lora_B, dtype=np.float32)
    router_w = np.asarray(router_w, dtype=np.float32)
    scalings = np.asarray(scalings, dtype=np.float32)

    # shared (replicated) tensors
    w8 = np.ascontiguousarray(
        (base_w * WS).reshape(OT, P, KO, P).transpose(0, 3, 2, 1)
    ).astype(f8)                                                   # [ot,p,ko,m]
    s_rep = np.repeat(scalings, RANK)                              # [128]
    aprime = (lora_A * s_rep[:, None]).astype(np.float32)          # [R, D]
    aa = np.ascontiguousarray(
        aprime.T.reshape(KO, P, R).transpose(1, 0, 2)
    ).astype(ml_dtypes.bfloat16)
    blo = np.ascontiguousarray(
        (lora_B * WS).reshape(OT, P, R).transpose(2, 0, 1)
    ).astype(ml_dtypes.bfloat16)                                   # [r,ot,m]
    rt = np.ascontiguousarray(router_w.T.reshape(KO, P, E).transpose(1, 0, 2))
    bias = np.ascontiguousarray(base_b.reshape(OT, P).T)           # [p,ot]
    e8 = np.zeros((E, P), dtype=np.float32)
    for e in range(E):
        e8[e, e * RANK : (e + 1) * RANK] = 1.0
    idm = np.eye(P, dtype=np.float32)

    in_maps = []
    for c in range(N_CORES):
        x_pc = x[c * T_PC : (c + 1) * T_PC]                        # [T_PC, D]
        x8 = np.ascontiguousarray(
            x_pc.T.reshape(KO, P, T_PC).transpose(1, 0, 2)
        ).astype(f8)                                               # [p,ko,t]
        xf = np.ascontiguousarray(
            x_pc.reshape(NGT, GT, KO, P).transpose(0, 3, 2, 1)
        )                                                          # [g,p,ko,u]
        xb = xf.astype(ml_dtypes.bfloat16)
        in_maps.append(
            {
                "x8": x8,
                "xf": xf,
                "xb": xb,
                "w8": w8,
                "aa": aa,
                "blo": blo,
                "rt": rt,
                "bias": bias,
                "e8": e8,
                "idm": idm,
            }
        )
    return in_maps


def assemble_output(results):
    """Per-core yo [OT, NT, P, TTILE] bf16 -> full [B, S, O] fp32."""
    outs = []
    for r in results:
        yo = np.asarray(r["yo"])                                   # bf16
        y = yo.transpose(1, 3, 0, 2).reshape(T_PC, O).astype(np.float32)
        outs.append(y)
    return np.concatenate(outs, axis=0).reshape(B, S, O)


def kernel(**inputs):
    _ensure_path()
    from concourse.bass_utils import run_bass_kernel_spmd

    assert int(inputs["top_k"]) == 2
    nc = get_program()
    in_maps = make_in_maps(
        inputs["x"],
        inputs["base_w"],
        inputs["base_b"],
        inputs["lora_A"],
        inputs["lora_B"],
        inputs["router_w"],
        inputs["scalings"],
    )
    res = run_bass_kernel_spmd(nc, in_maps, list(range(N_CORES)))
    return assemble_output(res.results)


if __name__ == "__main__":
    get_program()
    print("program built OK")


# revision 36
# speedup vs baseline: 1.1519x; 1.1519x over previous
"""Trainium2 Bass kernel for nn_GatedLinear (gated LoRA-MoE linear layer).

Math (see reference):
  base_out = x @ base_w.T + base_b
  logits   = x @ router_w.T ; top-2 softmax -> dense per-expert gate
  h        = x @ lora_A.T   ; rank_w = repeat(gate*scalings, 16)
  out      = base_out + (h * rank_w) @ lora_B.T

Sharding: pure data-parallel over batch*seq across 8 cores (1024 tokens
per core); all weights replicated. No collectives.

Implementation notes:
- The dominant base matmul runs in fp8 e4m3 with DoubleRow perf mode
  (2 k-subtiles contracted per instruction at 0.5 cycles/row). base_w is
  prescaled by 64 on the host so its values sit in e4m3's normal range;
  the 1/64 is folded into the bias epilogue. lora_B is prescaled by 64
  too so its f32r accumulation step shares the same PSUM scale.
  Measured end-to-end rel err of this scheme on the reference seed is
  ~1.1e-2 (tolerance 2e-2).
- The router must match fp32 top-2 selection exactly, so logits use
  true-fp32 matmuls on a separate fp32 copy of x, streamed per 256-token
  gating tile. The lora path (lora_A/lora_B matmuls, gated activations)
  runs in bf16 (same PE rate as f32r, half the SBUF/DMA, ~2e-3 error),
  fed by a second small bf16 x stream.
- All DRAM layouts are host-swizzled so every DMA is large-contiguous
  per partition (the previous version streamed base_w as 512-byte
  descriptors, starving the PE).
- Router/gating work for gating tile g+1 is interleaved with the gate
  expansion of tile g so the tensor engine never waits on the vector
  engine's top-k chain; the fused per-output-tile loop follows.
- Output is written bf16 in a DMA-friendly layout and de-swizzled/upcast
  on the host.
"""

import numpy as np
import ml_dtypes


def _ensure_path():
    try:
        import concourse.bass  # noqa: F401
    except ImportError:
        import sys

        for p in ("/opt/trn_rl_repo", "/root/.axon_site/_ro/trn_rl_repo"):
            if p not in sys.path:
                sys.path.insert(0, p)


N_CORES = 8
B, S, D, O = 4, 2048, 4096, 4096
T = B * S              # 8192 tokens total
T_PC = T // N_CORES    # 1024 tokens per core
E = 8                  # experts
RANK = 16
R = E * RANK           # 128 fused rank dim
P = 128
KO = D // P            # 32 k-subtiles of the contraction dim
KO2 = KO // 2          # 16 DoubleRow k-pairs
OT = O // P            # 32 output-feature tiles
TTILE = 512            # tokens per base-matmul moving operand
NT = T_PC // TTILE     # 2 token tiles per core
GT = 256               # gating token-tile size
NGT = T_PC // GT       # 4 gating tiles per core
NGC = GT // P          # 128-token chunks per gating tile
WS = 64.0              # fp8 weight prescale

_prog_cache = {}


def _build_program():
    """Build the single-core SPMD Bass program (same on all 8 cores)."""
    _ensure_path()
    import concourse.mybir as mybir
    import concourse.tile as tile
    from concourse import bacc

    f32 = mybir.dt.float32
    f32r = mybir.dt.float32r
    f8 = mybir.dt.float8e4
    bf16 = mybir.dt.bfloat16
    Alu = mybir.AluOpType
    Act = mybir.ActivationFunctionType
    DRow = mybir.MatmulPerfMode.DoubleRow

    nc = bacc.Bacc(
        "TRN2",
        target_bir_lowering=False,
        debug=False,
        num_devices=N_CORES,
    )

    x8d = nc.dram_tensor("x8", [P, KO, T_PC], f8, kind="ExternalInput").ap()
    xfd = nc.dram_tensor("xf", [NGT, P, KO, GT], f32, kind="ExternalInput").ap()
    xbd = nc.dram_tensor("xb", [NGT, P, KO, GT], bf16, kind="ExternalInput").ap()
    w8d = nc.dram_tensor("w8", [OT, P, KO, P], f8, kind="ExternalInput").ap()
    aAd = nc.dram_tensor("aa", [P, KO, R], bf16, kind="ExternalInput").ap()
    bBd = nc.dram_tensor("blo", [P, OT, P], bf16, kind="ExternalInput").ap()
    rtd = nc.dram_tensor("rt", [P, KO, E], f32, kind="ExternalInput").ap()
    bbd = nc.dram_tensor("bias", [P, OT], f32, kind="ExternalInput").ap()
    e8d = nc.dram_tensor("e8", [E, P], f32, kind="ExternalInput").ap()
    idd = nc.dram_tensor("idm", [P, P], f32, kind="ExternalInput").ap()
    yod = nc.dram_tensor("yo", [OT, NT, P, TTILE], bf16, kind="ExternalOutput").ap()

    from contextlib import ExitStack

    with tile.TileContext(nc) as tc:
        with (
            tc.tile_pool(name="pp", bufs=1) as pp,
            tc.tile_pool(name="xfp", bufs=2) as xfp,
            tc.tile_pool(name="wp", bufs=3) as wp,
            tc.tile_pool(name="gp", bufs=2) as gp,
            tc.tile_pool(name="ob", bufs=3) as ob,
        ):
            # gating-phase PSUM pools; closed before the fused loop so its
            # accumulator pool can reuse the banks (8-bank budget)
            phase1 = ExitStack()
            ps_lg = phase1.enter_context(
                tc.tile_pool(name="ps_lg", bufs=1, space="PSUM")
            )
            ps_h = phase1.enter_context(
                tc.tile_pool(name="ps_h", bufs=3, space="PSUM")
            )
            ps_sm = phase1.enter_context(
                tc.tile_pool(name="ps_sm", bufs=1, space="PSUM")
            )

            # ---- resident constants; small ones on the scalar queue ----
            rtsb = pp.tile([P, KO, E], f32)
            nc.scalar.dma_start(rtsb[:], rtd[:])
            aAsb = pp.tile([P, KO, R], bf16)
            nc.scalar.dma_start(aAsb[:], aAd[:])
            bbsb = pp.tile([P, OT], f32)
            nc.scalar.dma_start(bbsb[:], bbd[:])
            e8sb = pp.tile([E, P], f32)
            nc.scalar.dma_start(e8sb[:], e8d[:])
            idsb = pp.tile([P, P], f32)
            nc.scalar.dma_start(idsb[:], idd[:])
            bBsb = pp.tile([P, OT, P], bf16)  # DMA'd mid-gating (decongest)

            hwsb = pp.tile([P, T_PC], bf16)  # gated rank activations [r, t]
            x8sb = pp.tile([P, KO, T_PC], f8)

            # gpsimd queue: first two gating x-slices (fp32 + bf16), then
            # the fp8 x, then the rest (pool pacing keeps the queue from
            # running far ahead; x8 is ordered before xf[2] so it is not
            # blocked by the xf pool semaphore).
            xft = [None] * NGT
            xbt = [None] * NGT

            def issue_xf(g, chunks=1, bufs=3):
                xft[g] = xfp.tile(
                    [P, KO, GT], f32, tag="xf", name=f"xf{g}", bufs=bufs
                )
                # finer chunks give finer dep granularity: the first logits
                # matmuls can start as soon as their ko-slices arrive
                ck = KO // chunks
                for c in range(chunks):
                    nc.gpsimd.dma_start(
                        xft[g][:, c * ck : (c + 1) * ck, :],
                        xfd[g, :, c * ck : (c + 1) * ck, :],
                    )

            def issue_xb(g):
                xbt[g] = xfp.tile(
                    [P, KO, GT], bf16, tag="xb", name=f"xb{g}", bufs=2
                )
                nc.gpsimd.dma_start(xbt[g][:], xbd[g])

            issue_xf(0, chunks=4)
            issue_xb(0)
            issue_xf(1)
            issue_xb(1)
            issue_xf(2)
            issue_xb(2)
            issue_xf(3)
            issue_xb(3)
            # x8 is only needed by the fused loop (~120us in); keep it
            # behind the gating-phase x streams
            nc.gpsimd.dma_start(x8sb[:], x8d[:])

            # ---- gating round helpers (issue-order is the schedule) ----
            def issue_lg(g):
                """Router logits for gating tile g: exact fp32, expert-major.

                router_w is the stationary operand (only 8 columns, so
                LDWEIGHTS is ~7ns and hides); a stationary-x variant was
                measured slower (each 128-col fp32 weight load cannot hide
                behind an 8-row moving pass).
                """
                lg = ps_lg.tile([E, GT], f32, tag="lg")
                for ko in range(KO):
                    nc.tensor.matmul(
                        lg[:],
                        lhsT=rtsb[:, ko, :],
                        rhs=xft[g][:, ko, :],
                        start=(ko == 0),
                        stop=(ko == KO - 1),
                    )
                lgs = gp.tile([E, GT], f32, tag="lgs")
                nc.vector.tensor_copy(lgs[:], lg[:])
                return lgs

            def issue_tpfw(g, lgs):
                """Transpose logits to token-major [tok, chunk, expert]."""
                ltk = gp.tile([P, NGC, E], f32, tag="ltk")
                tp = ps_sm.tile([P, NGC, E], f32, tag="tp")
                for c in range(NGC):
                    nc.tensor.transpose(
                        tp[:, c, :], lgs[:, c * P : (c + 1) * P], idsb[:E, :E]
                    )
                nc.vector.tensor_copy(ltk[:], tp[:])
                return ltk

            def issue_h(g):
                """lora_A rank activations for tile g (bf16)."""
                hps = ps_h.tile([P, GT], f32, tag="h")
                for ko in range(KO):
                    nc.tensor.matmul(
                        hps[:],
                        lhsT=aAsb[:, ko, :],
                        rhs=xbt[g][:, ko, :],
                        start=(ko == 0),
                        stop=(ko == KO - 1),
                    )
                return hps

            def issue_topk(g, ltk):
                """Top-2 + softmax along the expert axis (DVE only)."""
                m1 = gp.tile([P, NGC, 1], f32, tag="m1")
                nc.vector.tensor_reduce(m1[:], ltk[:], mybir.AxisListType.X, Alu.max)
                mask1 = gp.tile([P, NGC, E], f32, tag="mask1")
                nc.vector.tensor_tensor(
                    mask1[:], ltk[:], m1.to_broadcast((P, NGC, E)), Alu.is_equal
                )
                l2 = gp.tile([P, NGC, E], f32, tag="l2")
                nc.vector.scalar_tensor_tensor(
                    l2[:], mask1[:], -1e30, ltk[:], Alu.mult, Alu.add
                )
                m2 = gp.tile([P, NGC, 1], f32, tag="m2")
                nc.vector.tensor_reduce(m2[:], l2[:], mybir.AxisListType.X, Alu.max)
                mask2 = gp.tile([P, NGC, E], f32, tag="mask2")
                nc.vector.tensor_tensor(
                    mask2[:], l2[:], m2.to_broadcast((P, NGC, E)), Alu.is_equal
                )
                dlt = gp.tile([P, NGC, 1], f32, tag="dlt")
                nc.vector.tensor_tensor(dlt[:], m2[:], m1[:], Alu.subtract)
                g2 = gp.tile([P, NGC, 1], f32, tag="g2")
                nc.scalar.activation(g2[:], dlt[:], Act.Sigmoid)
                g1 = gp.tile([P, NGC, 1], f32, tag="g1")
                nc.vector.tensor_scalar(g1[:], g2[:], -1.0, 1.0, Alu.mult, Alu.add)
                gate = gp.tile([P, NGC, E], f32, tag="gate", bufs=3)
                nc.vector.tensor_tensor(
                    gate[:], mask1[:], g1.to_broadcast((P, NGC, E)), Alu.mult
                )
                gm2 = gp.tile([P, NGC, E], f32, tag="gm2")
                nc.vector.tensor_tensor(
                    gm2[:], mask2[:], g2.to_broadcast((P, NGC, E)), Alu.mult
                )
                nc.vector.tensor_tensor(gate[:], gate[:], gm2[:], Alu.add)
                return gate

            def issue_expand(g, gate, hps):
                """Gates back to expert-major, expand to rank slots, gate h."""
                ts = slice(g * GT, (g + 1) * GT)
                gts = gp.tile([E, NGC, P], f32, tag="gts")
                tp2 = ps_sm.tile([E, NGC, P], f32, tag="tp2")
                for c in range(NGC):
                    nc.tensor.transpose(tp2[:, c, :], gate[:, c, :], idsb[:])
                nc.vector.tensor_copy(gts[:], tp2[:])
                RG = ps_sm.tile([P, GT], f32, tag="rg")
                nc.tensor.matmul(
                    RG[:], lhsT=e8sb[:], rhs=gts[:], start=True, stop=True
                )
                rgs = gp.tile([P, GT], f32, tag="rgs")
                nc.vector.tensor_copy(rgs[:], RG[:])
                nc.vector.tensor_tensor(hwsb[:, ts], hps[:], rgs[:], Alu.mult)

            # ---- gating rounds, software-pipelined ----
            # Per round: PE does lg(g), h(g), tpfw(g); the expand (tpbk,
            # e8, hw-gating) of round g-2 follows, two rounds behind, so
            # no PE op ever waits on the DVE top-k chain (measured
            # ~2.4us/round stall with a one-round lag). Mid-round we also
            # kick DMAs that are only needed later (lora_B, first base_w
            # tiles) to keep the startup window clear for the x streams.
            hps = [None] * NGT
            gates = [None] * NGT
            wtiles = []
            for g in range(NGT):
                lgsg = issue_lg(g)
                hps[g] = issue_h(g)
                ltkg = issue_tpfw(g, lgsg)
                gates[g] = issue_topk(g, ltkg)
                if g == 1:
                    nc.scalar.dma_start(bBsb[:], bBd[:])
                if g == 2:
                    for k in range(3):
                        wsb = wp.tile([P, KO, P], f8, tag="w", name=f"wpre{k}")
                        nc.scalar.dma_start(wsb[:], w8d[k])
                        wtiles.append(wsb)
                if g >= 2:
                    issue_expand(g - 2, gates[g - 2], hps[g - 2])
            issue_expand(NGT - 2, gates[NGT - 2], hps[NGT - 2])
            issue_expand(NGT - 1, gates[NGT - 1], hps[NGT - 1])
            phase1.close()

            # ---- fused base + lora output loop ----
            with tc.tile_pool(name="ps_acc", bufs=2, space="PSUM") as ps_acc:
                for ot in range(OT):
                    if ot < len(wtiles):
                        wsb = wtiles[ot]
                    else:
                        wsb = wp.tile([P, KO, P], f8, tag="w")
                        nc.sync.dma_start(wsb[:], w8d[ot])
                    for tt in range(NT):
                        ts = slice(tt * TTILE, (tt + 1) * TTILE)
                        acc = ps_acc.tile([P, TTILE], f32, tag="acc")
                        for k2 in range(KO2):
                            nc.tensor.matmul(
                                acc[:],
                                lhsT=wsb[:, 2 * k2 : 2 * k2 + 2, :],
                                rhs=x8sb[:, 2 * k2 : 2 * k2 + 2, ts],
                                start=(k2 == 0),
                                stop=False,
                                perf_mode=DRow,
                            )
                        nc.tensor.matmul(
                            acc[:],
                            lhsT=bBsb[:, ot, :],
                            rhs=hwsb[:, ts],
                            start=False,
                            stop=True,
                        )
                        osb = ob.tile([P, TTILE], bf16, tag="osb")
                        nc.vector.scalar_tensor_tensor(
                            osb[:],
                            acc[:],
                            1.0 / WS,
                            bbsb[:, ot, None].to_broadcast((P, TTILE)),
                            Alu.mult,
                            Alu.add,
                        )
                        nc.scalar.dma_start(yod[ot, tt], osb[:])

    nc.compile()
    return nc


def get_program():
    if "nc" not in _prog_cache:
        _prog_cache["nc"] = _build_program()
    return _prog_cache["nc"]


def make_in_maps(x, base_w, base_b, lora_A, lora_B, router_w, scalings):
    """Host-side sharding/layout prep -> per-core input dicts."""
    f8 = ml_dtypes.float8_e4m3
    x = np.ascontiguousarray(np.asarray(x, dtype=np.float32).reshape(T, D))
    base_w = np.asarray(base_w, dtype=np.float32)
    base_b = np.asarray(base_b, dtype=np.float32)
    lora_A = np.asarray(lora_A, dtype=np.float32)
    lora_B = np.asarray(lora_B, dtype=np.float32)
    router_w = np.asarray(router_w, dtype=np.float32)
    scalings = np.asarray(scalings, dtype=np.float32)

    # shared (replicated) tensors
    w8 = np.ascontiguousarray(
        (base_w * WS).reshape(OT, P, KO, P).transpose(0, 3, 2, 1)
    ).astype(f8)                                                   # [ot,p,ko,m]
    s_rep = np.repeat(scalings, RANK)                              # [128]
    aprime = (lora_A * s_rep[:, None]).astype(np.float32)          # [R, D]
    aa = np.ascontiguousarray(
        aprime.T.reshape(KO, P, R).transpose(1, 0, 2)
    ).astype(ml_dtypes.bfloat16)
    blo = np.ascontiguousarray(
        (lora_B * WS).reshape(OT, P, R).transpose(2, 0, 1)
    ).astype(ml_dtypes.bfloat16)                                   # [r,ot,m]
    rt = np.ascontiguousarray(router_w.T.reshape(KO, P, E).transpose(1, 0, 2))
    bias = np.ascontiguousarray(base_b.reshape(OT, P).T)           # [p,ot]
    e8 = np.zeros((E, P), dtype=np.float32)
    for e in range(E):
        e8[e, e * RANK : (e + 1) * RANK] = 1.0
    idm = np.eye(P, dtype=np.float32)

    in_maps = []
    for c in range(N_CORES):
        x_pc = x[c * T_PC : (c + 1) * T_PC]                        # [T_PC, D]
        x8 = np.ascontiguousarray(
            x_pc.T.reshape(KO, P, T_PC).transpose(1, 0, 2)
        ).astype(f8)                                               # [p,ko,t]
        xf = np.ascontiguousarray(
            x_pc.reshape(NGT, GT, KO, P).transpose(0, 3, 2, 1)
        )                                                          # [g,p,ko,u]
        xb = xf.astype(ml_dtypes.bfloat16)
        in_maps.append(
            {
                "x8": x8,
                "xf": xf,
                "xb": xb,
                "w8": w8,
                "aa": aa,
                "blo": blo,
                "rt": rt,
                "bias": bias,
                "e8": e8,
                "idm": idm,
            }
        )
    return in_maps


def assemble_output(results):
    """Per-core yo [OT, NT, P, TTILE] bf16 -> full [B, S, O] fp32."""
    outs = []
    for r in results:
        yo = np.asarray(r["yo"])                                   # bf16
        y = yo.transpose(1, 3, 0, 2).reshape(T_PC, O).astype(np.float32)
        outs.append(y)
    return np.concatenate(outs, axis=0).reshape(B, S, O)


def kernel(**inputs):
    _ensure_path()
    from concourse.bass_utils import run_bass_kernel_spmd

    assert int(inputs["top_k"]) == 2
    nc = get_program()
    in_maps = make_in_maps(
        inputs["x"],
        inputs["base_w"],
        inputs["base_b"],
        inputs["lora_A"],
        inputs["lora_B"],
        inputs["router_w"],
        inputs["scalings"],
    )
    res = run_bass_kernel_spmd(nc, in_maps, list(range(N_CORES)))
    return assemble_output(res.results)


if __name__ == "__main__":
    get_program()
    print("program built OK")


# revision 40
# speedup vs baseline: 1.1878x; 1.0312x over previous
"""Trainium2 Bass kernel for nn_GatedLinear (gated LoRA-MoE linear layer).

Math (see reference):
  base_out = x @ base_w.T + base_b
  logits   = x @ router_w.T ; top-2 softmax -> dense per-expert gate
  h        = x @ lora_A.T   ; rank_w = repeat(gate*scalings, 16)
  out      = base_out + (h * rank_w) @ lora_B.T

Sharding: pure data-parallel over batch*seq across 8 cores (1024 tokens
per core); all weights replicated. No collectives.

Implementation notes:
- The dominant base matmul runs in fp8 e4m3 with DoubleRow perf mode
  (2 k-subtiles contracted per instruction at 0.5 cycles/row). base_w is
  prescaled by 64 on the host so its values sit in e4m3's normal range;
  the 1/64 is folded into the bias epilogue. lora_B is prescaled by 64
  too so its f32r accumulation step shares the same PSUM scale.
  Measured end-to-end rel err of this scheme on the reference seed is
  ~1.1e-2 (tolerance 2e-2).
- The router must match fp32 top-2 selection exactly, so logits use
  true-fp32 matmuls on a separate fp32 copy of x, streamed per 256-token
  gating tile. The lora path (lora_A/lora_B matmuls, gated activations)
  runs in bf16 (same PE rate as f32r, half the SBUF/DMA, ~2e-3 error),
  fed by a second small bf16 x stream.
- All DRAM layouts are host-swizzled so every DMA is large-contiguous
  per partition (the previous version streamed base_w as 512-byte
  descriptors, starving the PE).
- Router/gating work for gating tile g+1 is interleaved with the gate
  expansion of tile g so the tensor engine never waits on the vector
  engine's top-k chain; the fused per-output-tile loop follows.
- Output is written bf16 in a DMA-friendly layout and de-swizzled/upcast
  on the host.
"""

import numpy as np
import ml_dtypes


def _ensure_path():
    try:
        import concourse.bass  # noqa: F401
    except ImportError:
        import sys

        for p in ("/opt/trn_rl_repo", "/root/.axon_site/_ro/trn_rl_repo"):
            if p not in sys.path:
                sys.path.insert(0, p)


N_CORES = 8
B, S, D, O = 4, 2048, 4096, 4096
T = B * S              # 8192 tokens total
T_PC = T // N_CORES    # 1024 tokens per core
E = 8                  # experts
RANK = 16
R = E * RANK           # 128 fused rank dim
P = 128
KO = D // P            # 32 k-subtiles of the contraction dim
KO2 = KO // 2          # 16 DoubleRow k-pairs
OT = O // P            # 32 output-feature tiles
TTILE = 512            # tokens per base-matmul moving operand
NT = T_PC // TTILE     # 2 token tiles per core
GT = 256               # gating token-tile size
NGT = T_PC // GT       # 4 gating tiles per core
NGC = GT // P          # 128-token chunks per gating tile
WS = 64.0              # fp8 weight prescale

_prog_cache = {}


def _build_program():
    """Build the single-core SPMD Bass program (same on all 8 cores)."""
    _ensure_path()
    import concourse.mybir as mybir
    import concourse.tile as tile
    from concourse import bacc

    f32 = mybir.dt.float32
    f32r = mybir.dt.float32r
    f8 = mybir.dt.float8e4
    bf16 = mybir.dt.bfloat16
    Alu = mybir.AluOpType
    Act = mybir.ActivationFunctionType
    DRow = mybir.MatmulPerfMode.DoubleRow

    nc = bacc.Bacc(
        "TRN2",
        target_bir_lowering=False,
        debug=False,
        num_devices=N_CORES,
    )

    xfd = nc.dram_tensor("xf", [NGT, P, KO, GT], f32, kind="ExternalInput").ap()
    w8d = nc.dram_tensor("w8", [OT, P, KO, P], f8, kind="ExternalInput").ap()
    aAd = nc.dram_tensor("aa", [P, KO, R], bf16, kind="ExternalInput").ap()
    bBd = nc.dram_tensor("blo", [P, OT, P], bf16, kind="ExternalInput").ap()
    rtd = nc.dram_tensor("rt", [P, KO, E], f32, kind="ExternalInput").ap()
    bbd = nc.dram_tensor("bias", [P, OT], f32, kind="ExternalInput").ap()
    e8d = nc.dram_tensor("e8", [E, P], f32, kind="ExternalInput").ap()
    idd = nc.dram_tensor("idm", [P, P], f32, kind="ExternalInput").ap()
    yod = nc.dram_tensor("yo", [OT, NT, P, TTILE], bf16, kind="ExternalOutput").ap()

    from contextlib import ExitStack

    with tile.TileContext(nc) as tc:
        with (
            tc.tile_pool(name="pp", bufs=1) as pp,
            tc.tile_pool(name="xfp", bufs=2) as xfp,
            tc.tile_pool(name="wp", bufs=3) as wp,
            tc.tile_pool(name="gp", bufs=2) as gp,
            tc.tile_pool(name="ob", bufs=3) as ob,
        ):
            # gating-phase PSUM pools; closed before the fused loop so its
            # accumulator pool can reuse the banks (8-bank budget)
            phase1 = ExitStack()
            ps_lg = phase1.enter_context(
                tc.tile_pool(name="ps_lg", bufs=1, space="PSUM")
            )
            ps_h = phase1.enter_context(
                tc.tile_pool(name="ps_h", bufs=3, space="PSUM")
            )
            ps_sm = phase1.enter_context(
                tc.tile_pool(name="ps_sm", bufs=1, space="PSUM")
            )

            # ---- resident constants; small ones on the scalar queue ----
            rtsb = pp.tile([P, KO, E], f32)
            nc.scalar.dma_start(rtsb[:], rtd[:])
            aAsb = pp.tile([P, KO, R], bf16)
            nc.scalar.dma_start(aAsb[:], aAd[:])
            bbsb = pp.tile([P, OT], f32)
            nc.scalar.dma_start(bbsb[:], bbd[:])
            e8sb = pp.tile([E, P], f32)
            nc.scalar.dma_start(e8sb[:], e8d[:])
            idsb = pp.tile([P, P], f32)
            nc.scalar.dma_start(idsb[:], idd[:])
            bBsb = pp.tile([P, OT, P], bf16)  # DMA'd mid-gating (decongest)

            hwsb = pp.tile([P, T_PC], bf16)  # gated rank activations [r, t]
            x8sb = pp.tile([P, KO, T_PC], f8)

            # Only the fp32 x stream is DMA'd; the bf16 (lora) and fp8
            # (base) copies are derived on-device — DVE and the scalar
            # engine are mostly idle, and this removes 12.6MB/core of DMA
            # that was stalling the gating rounds.
            xft = [None] * NGT
            xbt = [None] * NGT

            def issue_xf(g, chunks=1, bufs=3):
                xft[g] = xfp.tile(
                    [P, KO, GT], f32, tag="xf", name=f"xf{g}", bufs=bufs
                )
                # finer chunks give finer dep granularity: the first logits
                # matmuls can start as soon as their ko-slices arrive
                ck = KO // chunks
                for c in range(chunks):
                    nc.gpsimd.dma_start(
                        xft[g][:, c * ck : (c + 1) * ck, :],
                        xfd[g, :, c * ck : (c + 1) * ck, :],
                    )

            def conv_xb(g):
                """bf16 copy of x tile g for the lora_A matmul (DVE)."""
                xbt[g] = xfp.tile(
                    [P, KO, GT], bf16, tag="xb", name=f"xb{g}", bufs=2
                )
                nc.vector.tensor_copy(xbt[g][:], xft[g][:])

            def conv_x8(g):
                """e4m3 slice of x tile g for the base matmul (scalar)."""
                ts = slice(g * GT, (g + 1) * GT)
                nc.scalar.activation(x8sb[:, :, ts], xft[g][:], Act.Copy)

            for g in range(NGT):
                issue_xf(g, chunks=(4 if g == 0 else 1))
            conv_xb(0)
            conv_x8(0)

            # ---- gating round helpers (issue-order is the schedule) ----
            def issue_lg(g):
                """Router logits for gating tile g: exact fp32, expert-major.

                router_w is the stationary operand (only 8 columns, so
                LDWEIGHTS is ~7ns and hides); a stationary-x variant was
                measured slower (each 128-col fp32 weight load cannot hide
                behind an 8-row moving pass).
                """
                lg = ps_lg.tile([E, GT], f32, tag="lg")
                for ko in range(KO):
                    nc.tensor.matmul(
                        lg[:],
                        lhsT=rtsb[:, ko, :],
                        rhs=xft[g][:, ko, :],
                        start=(ko == 0),
                        stop=(ko == KO - 1),
                    )
                lgs = gp.tile([E, GT], f32, tag="lgs")
                nc.vector.tensor_copy(lgs[:], lg[:])
                return lgs

            def issue_tpfw(g, lgs):
                """Transpose logits to token-major [tok, chunk, expert]."""
                ltk = gp.tile([P, NGC, E], f32, tag="ltk")
                tp = ps_sm.tile([P, NGC, E], f32, tag="tp")
                for c in range(NGC):
                    nc.tensor.transpose(
                        tp[:, c, :], lgs[:, c * P : (c + 1) * P], idsb[:E, :E]
                    )
                nc.vector.tensor_copy(ltk[:], tp[:])
                return ltk

            def issue_h(g):
                """lora_A rank activations for tile g (bf16)."""
                hps = ps_h.tile([P, GT], f32, tag="h")
                for ko in range(KO):
                    nc.tensor.matmul(
                        hps[:],
                        lhsT=aAsb[:, ko, :],
                        rhs=xbt[g][:, ko, :],
                        start=(ko == 0),
                        stop=(ko == KO - 1),
                    )
                return hps

            def issue_topk(g, ltk):
                """Top-2 + softmax along the expert axis (DVE only)."""
                m1 = gp.tile([P, NGC, 1], f32, tag="m1")
                nc.vector.tensor_reduce(m1[:], ltk[:], mybir.AxisListType.X, Alu.max)
                mask1 = gp.tile([P, NGC, E], f32, tag="mask1")
                nc.vector.tensor_tensor(
                    mask1[:], ltk[:], m1.to_broadcast((P, NGC, E)), Alu.is_equal
                )
                l2 = gp.tile([P, NGC, E], f32, tag="l2")
                nc.vector.scalar_tensor_tensor(
                    l2[:], mask1[:], -1e30, ltk[:], Alu.mult, Alu.add
                )
                m2 = gp.tile([P, NGC, 1], f32, tag="m2")
                nc.vector.tensor_reduce(m2[:], l2[:], mybir.AxisListType.X, Alu.max)
                mask2 = gp.tile([P, NGC, E], f32, tag="mask2")
                nc.vector.tensor_tensor(
                    mask2[:], l2[:], m2.to_broadcast((P, NGC, E)), Alu.is_equal
                )
                dlt = gp.tile([P, NGC, 1], f32, tag="dlt")
                nc.vector.tensor_tensor(dlt[:], m2[:], m1[:], Alu.subtract)
                g2 = gp.tile([P, NGC, 1], f32, tag="g2")
                nc.scalar.activation(g2[:], dlt[:], Act.Sigmoid)
                g1 = gp.tile([P, NGC, 1], f32, tag="g1")
                nc.vector.tensor_scalar(g1[:], g2[:], -1.0, 1.0, Alu.mult, Alu.add)
                gate = gp.tile([P, NGC, E], f32, tag="gate", bufs=3)
                nc.vector.tensor_tensor(
                    gate[:], mask1[:], g1.to_broadcast((P, NGC, E)), Alu.mult
                )
                gm2 = gp.tile([P, NGC, E], f32, tag="gm2")
                nc.vector.tensor_tensor(
                    gm2[:], mask2[:], g2.to_broadcast((P, NGC, E)), Alu.mult
                )
                nc.vector.tensor_tensor(gate[:], gate[:], gm2[:], Alu.add)
                return gate

            def issue_expand(g, gate, hps):
                """Gates back to expert-major, expand to rank slots, gate h."""
                ts = slice(g * GT, (g + 1) * GT)
                gts = gp.tile([E, NGC, P], f32, tag="gts")
                tp2 = ps_sm.tile([E, NGC, P], f32, tag="tp2")
                for c in range(NGC):
                    nc.tensor.transpose(tp2[:, c, :], gate[:, c, :], idsb[:])
                nc.vector.tensor_copy(gts[:], tp2[:])
                RG = ps_sm.tile([P, GT], f32, tag="rg")
                nc.tensor.matmul(
                    RG[:], lhsT=e8sb[:], rhs=gts[:], start=True, stop=True
                )
                rgs = gp.tile([P, GT], f32, tag="rgs")
                nc.vector.tensor_copy(rgs[:], RG[:])
                nc.vector.tensor_tensor(hwsb[:, ts], hps[:], rgs[:], Alu.mult)

            # ---- gating rounds, software-pipelined ----
            # Per round: PE does lg(g), h(g), tpfw(g); the expand (tpbk,
            # e8, hw-gating) of round g-2 follows, two rounds behind, so
            # no PE op ever waits on the DVE top-k chain (measured
            # ~2.4us/round stall with a one-round lag). Mid-round we also
            # kick DMAs that are only needed later (lora_B, first base_w
            # tiles) to keep the startup window clear for the x streams.
            hps = [None] * NGT
            gates = [None] * NGT
            wtiles = []
            for g in range(NGT):
                if g + 1 < NGT:
                    conv_xb(g + 1)
                    conv_x8(g + 1)
                lgsg = issue_lg(g)
                hps[g] = issue_h(g)
                ltkg = issue_tpfw(g, lgsg)
                gates[g] = issue_topk(g, ltkg)
                if g == 1:
                    nc.scalar.dma_start(bBsb[:], bBd[:])
                if g == 2:
                    for k in range(3):
                        wsb = wp.tile([P, KO, P], f8, tag="w", name=f"wpre{k}")
                        nc.scalar.dma_start(wsb[:], w8d[k])
                        wtiles.append(wsb)
                if g >= 2:
                    issue_expand(g - 2, gates[g - 2], hps[g - 2])
            issue_expand(NGT - 2, gates[NGT - 2], hps[NGT - 2])
            issue_expand(NGT - 1, gates[NGT - 1], hps[NGT - 1])
            phase1.close()

            # ---- fused base + lora output loop ----
            with tc.tile_pool(name="ps_acc", bufs=2, space="PSUM") as ps_acc:
                for ot in range(OT):
                    if ot < len(wtiles):
                        wsb = wtiles[ot]
                    else:
                        wsb = wp.tile([P, KO, P], f8, tag="w")
                        nc.sync.dma_start(wsb[:], w8d[ot])
                    for tt in range(NT):
                        ts = slice(tt * TTILE, (tt + 1) * TTILE)
                        acc = ps_acc.tile([P, TTILE], f32, tag="acc")
                        for k2 in range(KO2):
                            nc.tensor.matmul(
                                acc[:],
                                lhsT=wsb[:, 2 * k2 : 2 * k2 + 2, :],
                                rhs=x8sb[:, 2 * k2 : 2 * k2 + 2, ts],
                                start=(k2 == 0),
                                stop=False,
                                perf_mode=DRow,
                            )
                        nc.tensor.matmul(
                            acc[:],
                            lhsT=bBsb[:, ot, :],
                            rhs=hwsb[:, ts],
                            start=False,
                            stop=True,
                        )
                        osb = ob.tile([P, TTILE], bf16, tag="osb")
                        nc.vector.scalar_tensor_tensor(
                            osb[:],
                            acc[:],
                            1.0 / WS,
                            bbsb[:, ot, None].to_broadcast((P, TTILE)),
                            Alu.mult,
                            Alu.add,
                        )
                        nc.scalar.dma_start(yod[ot, tt], osb[:])

    nc.compile()
    return nc


def get_program():
    if "nc" not in _prog_cache:
        _prog_cache["nc"] = _build_program()
    return _prog_cache["nc"]


def make_in_maps(x, base_w, base_b, lora_A, lora_B, router_w, scalings):
    """Host-side sharding/layout prep -> per-core input dicts."""
    f8 = ml_dtypes.float8_e4m3
    x = np.ascontiguousarray(np.asarray(x, dtype=np.float32).reshape(T, D))
    base_w = np.asarray(base_w, dtype=np.float32)
    base_b = np.asarray(base_b, dtype=np.float32)
    lora_A = np.asarray(lora_A, dtype=np.float32)
    lora_B = np.asarray(lora_B, dtype=np.float32)
    router_w = np.asarray(router_w, dtype=np.float32)
    scalings = np.asarray(scalings, dtype=np.float32)

    # shared (replicated) tensors
    w8 = np.ascontiguousarray(
        (base_w * WS).reshape(OT, P, KO, P).transpose(0, 3, 2, 1)
    ).astype(f8)                                                   # [ot,p,ko,m]
    s_rep = np.repeat(scalings, RANK)                              # [128]
    aprime = (lora_A * s_rep[:, None]).astype(np.float32)          # [R, D]
    aa = np.ascontiguousarray(
        aprime.T.reshape(KO, P, R).transpose(1, 0, 2)
    ).astype(ml_dtypes.bfloat16)
    blo = np.ascontiguousarray(
        (lora_B * WS).reshape(OT, P, R).transpose(2, 0, 1)
    ).astype(ml_dtypes.bfloat16)                                   # [r,ot,m]
    rt = np.ascontiguousarray(router_w.T.reshape(KO, P, E).transpose(1, 0, 2))
    bias = np.ascontiguousarray(base_b.reshape(OT, P).T)           # [p,ot]
    e8 = np.zeros((E, P), dtype=np.float32)
    for e in range(E):
        e8[e, e * RANK : (e + 1) * RANK] = 1.0
    idm = np.eye(P, dtype=np.float32)

    in_maps = []
    for c in range(N_CORES):
        x_pc = x[c * T_PC : (c + 1) * T_PC]                        # [T_PC, D]
        xf = np.ascontiguousarray(
            x_pc.reshape(NGT, GT, KO, P).transpose(0, 3, 2, 1)
        )                                                          # [g,p,ko,u]
        in_maps.append(
            {
                "xf": xf,
                "w8": w8,
                "aa": aa,
                "blo": blo,
                "rt": rt,
                "bias": bias,
                "e8": e8,
                "idm": idm,
            }
        )
    return in_maps


def assemble_output(results):
    """Per-core yo [OT, NT, P, TTILE] bf16 -> full [B, S, O] fp32."""
    outs = []
    for r in results:
        yo = np.asarray(r["yo"])                                   # bf16
        y = yo.transpose(1, 3, 0, 2).reshape(T_PC, O).astype(np.float32)
        outs.append(y)
    return np.concatenate(outs, axis=0).reshape(B, S, O)


def kernel(**inputs):
    _ensure_path()
    from concourse.bass_utils import run_bass_kernel_spmd

    assert int(inputs["top_k"]) == 2
    nc = get_program()
    in_maps = make_in_maps(
        inputs["x"],
        inputs["base_w"],
        inputs["base_b"],
        inputs["lora_A"],
        inputs["lora_B"],
        inputs["router_w"],
        inputs["scalings"],
    )
    res = run_bass_kernel_spmd(nc, in_maps, list(range(N_CORES)))
    return assemble_output(res.results)


if __name__ == "__main__":
    get_program()
    print("program built OK")
